# revision 14
# baseline (speedup 1.0000x reference)
"""Trainium2 Bass kernel v2 for nn_AttentionLayer (BS=16, NA=NB=2048, NK=NV=64).

reference:
    w    = softmax_over_NA(key @ query^T / sqrt(NK))      (BS, NA, NB)
    vals = einsum('ban,bav->bnv', w, value)               (BS, NB, NV)
    out  = layernorm_over_NV(vals).transpose(0, 2, 1)     (BS, NV, NB)
    returns (out, w)

Sharding: data-parallel over BS across 8 cores (2 batches/core).

Numerics / speed choices (v2):
  - mm1 in bf16 with hi/lo split (error ~1e-6): s = khi@qhi + [khi;klo]@[qlo;qhi]
    streams 1 col/cycle on the PE (f32/f32r moving operands are 2-4x slower).
  - e = exp(s/8) stored fp16: mm2 runs at 1 col/cycle, and the softmax
    divide w = e * recip runs in DVE 2x_1P mode (all-16-bit operands).
  - w tiles are fp16; the gpsimd (SWDGE) DMA casts fp16->f32 on store.
  - softmax denominators: taken from the PE-transposed vals_aug tail
    (per-partition layout), reciprocal on 128 lanes, bounced through DRAM
    to partition-broadcast back as an fp16 row-replicated tile.
  - layernorm over NV via bn_stats/bn_aggr after PE transpose; unbiased
    std via exp(0.5*ln(var*64/63)) keeps ScalarE on one table set
    (natural_log_exp_and_others, pinned by filtering the table list).
  - the softmax denominator cancels in the layernorm (scale invariance).
"""

import sys

for _p in ("/opt/trn_rl_repo", "/opt/trn_rl_repo/concourse"):
    if _p not in sys.path:
        sys.path.insert(0, _p)

from contextlib import ExitStack

import ml_dtypes
import numpy as np

import concourse.bass as bass
import concourse.tile as tile
from concourse import bacc, mybir
from concourse.bass_utils import run_bass_kernel_spmd

BS, NA, NB, NK, NV = 16, 2048, 2048, 64, 64
NCORES = 8
BPC = BS // NCORES        # batches per core
NBH = NB // 2             # NB half processed at a time
NAC = NA // 128           # number of 128-row NA chunks
F32 = mybir.dt.float32
BF16 = mybir.dt.bfloat16
FP16 = mybir.dt.float16
AF = mybir.ActivationFunctionType

TRACE = False             # test.py flips this to profile
_cache = {}


class _PinnedBacc(bacc.Bacc):
    """Bacc that pins all ScalarE table loads to natural_log_exp_and_others.

    The default per-function choice alternates between exp_and_others (for
    Exp) and natural_log_exp_and_others (for Ln), reloading tables ~2.7us
    each time. Emptying the competing sets (list order/indices preserved)
    forces one resident set.
    """

    def insert_act_table_loads(self):
        from concourse.hw_specs import get_activation_tables
        from concourse.bass import _bass_rust

        has_activation = any(
            isinstance(i, mybir.InstActivation)
            for b in self.main_func.blocks
            for i in b.instructions
        )
        if not has_activation:
            return
        used = {AF.Exp, AF.Ln, AF.Copy, AF.Identity, AF.Square}
        tables = []
        for name, fns in get_activation_tables(self.m.arch).items():
            if name != "natural_log_exp_and_others":
                fns = fns - used
            tables.append((name, fns))
        _bass_rust.insert_act_table_loads(self, tables)


def _build():
    nc = _PinnedBacc("TRN2", target_bir_lowering=False, debug=False,
                     num_devices=NCORES)
    ks_d = nc.dram_tensor("ks", [BPC, 128, NA], BF16, kind="ExternalInput")
    qa_d = nc.dram_tensor("qa", [BPC, 128, NB], BF16, kind="ExternalInput")
    qb_d = nc.dram_tensor("qb", [BPC, 128, NB], BF16, kind="ExternalInput")
    va_d = nc.dram_tensor("va", [BPC, NAC, 128, 128], FP16,
                          kind="ExternalInput")
    id_d = nc.dram_tensor("ident", [128, 128], F32, kind="ExternalInput")
    w_d = nc.dram_tensor("w_out", [BPC, NA, NB], F32, kind="ExternalOutput")
    o_d = nc.dram_tensor("o_out", [BPC, NB, NV], F32, kind="ExternalOutput")
    d_d = nc.dram_tensor("d_scratch", [BPC * 2, NBH], FP16)

    with ExitStack() as ctx:
        tc = ctx.enter_context(tile.TileContext(nc))
        consts = ctx.enter_context(tc.tile_pool(name="consts", bufs=1))
        inp = ctx.enter_context(tc.tile_pool(name="inp", bufs=2))
        epool = ctx.enter_context(tc.tile_pool(name="e", bufs=32))
        wpool = ctx.enter_context(tc.tile_pool(name="w", bufs=8))
        tailp = ctx.enter_context(tc.tile_pool(name="tail", bufs=2))
        small = ctx.enter_context(tc.tile_pool(name="small", bufs=4))
        ps_s = ctx.enter_context(tc.tile_pool(name="ps_s", bufs=2, space="PSUM"))
        ps_acc = ctx.enter_context(tc.tile_pool(name="ps_acc", bufs=1, space="PSUM"))
        ps_tp = ctx.enter_context(tc.tile_pool(name="ps_tp", bufs=2, space="PSUM"))

        ident = consts.tile([128, 128], F32)
        nc.sync.dma_start(out=ident, in_=id_d[:, :])

        # prefetch all batches' inputs upfront so later-batch loads are
        # not queued behind tail DMAs on the sync queue
        loaded = []
        for b in range(BPC):
            ks = inp.tile([128, NA], BF16, tag="ks")
            nc.sync.dma_start(out=ks, in_=ks_d[b])
            qa = inp.tile([128, NB], BF16, tag="qa")
            nc.sync.dma_start(out=qa, in_=qa_d[b])
            qb = inp.tile([128, NB], BF16, tag="qb")
            nc.sync.dma_start(out=qb, in_=qb_d[b])
            va = inp.tile([128, NAC, 128], FP16, tag="va")
            nc.sync.dma_start(out=va, in_=va_d[b].rearrange("c p v -> p c v"))
            loaded.append((ks, qa, qb, va))

        for b in range(BPC):
            ks, qa, qb, va = loaded[b]

            for h in range(2):
                hs = h * NBH
                acc = ps_acc.tile([128, NBH], F32, tag="acc")
                es = []
                for a in range(NAC):
                    s = ps_s.tile([128, NBH], F32, tag="s")
                    for q in range(2):
                        sl = slice(hs + q * 512, hs + (q + 1) * 512)
                        nc.tensor.matmul(
                            s[:, q * 512:(q + 1) * 512],
                            lhsT=ks[:, a * 128:(a + 1) * 128],
                            rhs=qa[:, sl],
                            start=True, stop=False)
                        nc.tensor.matmul(
                            s[:, q * 512:(q + 1) * 512],
                            lhsT=ks[:, a * 128:(a + 1) * 128],
                            rhs=qb[:, sl],
                            start=False, stop=True)
                    e = epool.tile([128, NBH], FP16, tag="e")
                    nc.scalar.activation(e, s, AF.Exp, scale=float(1.0 / np.sqrt(NK)))
                    for q in range(2):
                        nc.tensor.matmul(
                            acc[:, q * 512:(q + 1) * 512],
                            lhsT=va[:, a, :],
                            rhs=e[:, q * 512:(q + 1) * 512],
                            start=(a == 0), stop=(a == NAC - 1))
                    es.append(e)

                # ---- vals_aug tail: copy + transpose + stats + recip ----
                va_s = tailp.tile([128, NBH], F32, tag="vas")
                nc.scalar.copy(va_s, acc)

                bigts = tailp.tile([128, 8, NV + 1], F32, tag="bigts")
                mvs = small.tile([128, 8, 2], F32, tag="mvs")
                for j in range(8):
                    tp = ps_tp.tile([128, 128], F32, tag="tp")
                    nc.tensor.transpose(tp, va_s[:, j * 128:(j + 1) * 128],
                                        ident)
                    nc.vector.tensor_copy(bigts[:, j, :], tp[:, 0:NV + 1])
                    st = small.tile([128, 6], F32, tag="st")
                    nc.vector.bn_stats(st, bigts[:, j, 0:NV])
                    nc.vector.bn_aggr(mvs[:, j, :], st)

                # softmax denominators d sit in bigts[:, :, 64] with
                # column c = j*128 + p; reciprocal on all 128 lanes,
                # bounce through DRAM, broadcast back as fp16 rows.
                recv = small.tile([128, 8, 1], F32, tag="recv")
                nc.vector.reciprocal(recv, bigts[:, :, NV:NV + 1])
                recv16 = small.tile([128, 8, 1], FP16, tag="recv16")
                nc.vector.tensor_copy(recv16, recv)
                i_hb = b * 2 + h
                nc.gpsimd.dma_start(
                    out=d_d[i_hb:i_hb + 1, :].rearrange("o (j p) -> (o p) j", p=128),
                    in_=recv16[:, :, 0])
                rb = tailp.tile([128, NBH], FP16, tag="rb")
                nc.gpsimd.dma_start(
                    out=rb, in_=d_d[i_hb:i_hb + 1, :].partition_broadcast(128))

                # ---- w = e * recip (all fp16, DVE 2x), cast-store ----
                for g in range(4):
                    wt = wpool.tile([128, 4, NBH], FP16, tag="w")
                    for al in range(4):
                        nc.vector.tensor_mul(wt[:, al, :], es[g * 4 + al], rb)
                    nc.gpsimd.dma_start(
                        out=w_d[b, g * 512:(g + 1) * 512, hs:hs + NBH]
                            .rearrange("(a p) c -> p a c", p=128),
                        in_=wt)

                # ---- layernorm apply (d cancels by scale invariance) ----
                lt = small.tile([128, 8], F32, tag="lt")
                nc.scalar.activation(lt, mvs[:, :, 1], AF.Ln,
                                     scale=float(NV / (NV - 1)))
                sc = small.tile([128, 8], F32, tag="sc")
                nc.scalar.activation(sc, lt, AF.Exp, scale=-0.5)
                on = tailp.tile([128, 8, NV], F32, tag="on")
                for j in range(8):
                    nc.vector.tensor_scalar(
                        on[:, j, :], bigts[:, j, 0:NV],
                        scalar1=mvs[:, j, 0:1], scalar2=sc[:, j:j + 1],
                        op0=mybir.AluOpType.subtract,
                        op1=mybir.AluOpType.mult)
                nc.sync.dma_start(
                    out=o_d[b, hs:hs + NBH, :].rearrange("(j p) v -> p j v", p=128),
                    in_=on)

    nc.compile()
    return nc


def _prep_core_inputs(key, query, value, core):
    """Host-side shard + layout prep for one core (2 batches)."""
    b0 = core * BPC
    k = key[b0:b0 + BPC, :, 0, :]          # (BPC, NA, NK)
    q = query[b0:b0 + BPC, 0, :, :]        # (BPC, NB, NK)
    v = value[b0:b0 + BPC]                 # (BPC, NA, NV)
    kt = np.ascontiguousarray(k.transpose(0, 2, 1))   # (BPC, NK, NA) f32
    qt = np.ascontiguousarray(q.transpose(0, 2, 1))   # (BPC, NK, NB) f32

    khi = kt.astype(ml_dtypes.bfloat16)
    klo = (kt - khi.astype(np.float32)).astype(ml_dtypes.bfloat16)
    qhi = qt.astype(ml_dtypes.bfloat16)
    qlo = (qt - qhi.astype(np.float32)).astype(ml_dtypes.bfloat16)
    ks = np.concatenate([khi, klo], axis=1)           # (BPC, 128, NA)
    qa = np.concatenate([qhi, qhi], axis=1)           # (BPC, 128, NB)
    qb = np.concatenate(
        [qlo, np.zeros_like(qlo)], axis=1)            # (BPC, 128, NB)

    va = np.concatenate(
        [v, np.ones((BPC, NA, 1), np.float32),
         np.zeros((BPC, NA, 128 - NV - 1), np.float32)], axis=2)
    va = np.ascontiguousarray(
        va.reshape(BPC, NAC, 128, 128)).astype(np.float16)
    return {"ks": ks, "qa": qa, "qb": qb, "va": va,
            "ident": np.eye(128, dtype=np.float32)}


def kernel(key, query, value):
    key = np.asarray(key, dtype=np.float32)
    query = np.asarray(query, dtype=np.float32)
    value = np.asarray(value, dtype=np.float32)

    if "nc" not in _cache:
        _cache["nc"] = _build()
    nc = _cache["nc"]

    in_maps = [_prep_core_inputs(key, query, value, c) for c in range(NCORES)]
    res = run_bass_kernel_spmd(nc, in_maps, list(range(NCORES)), trace=TRACE)
    _cache["last_results"] = res

    w = np.concatenate([res.results[c]["w_out"] for c in range(NCORES)], axis=0)
    o_nbnv = np.concatenate([res.results[c]["o_out"] for c in range(NCORES)],
                            axis=0)                    # (BS, NB, NV)
    out = np.ascontiguousarray(o_nbnv.transpose(0, 2, 1))  # (BS, NV, NB)
    return out, w


# revision 15
# speedup vs baseline: 1.2642x; 1.2642x over previous
"""Trainium2 Bass kernel v2 for nn_AttentionLayer (BS=16, NA=NB=2048, NK=NV=64).

reference:
    w    = softmax_over_NA(key @ query^T / sqrt(NK))      (BS, NA, NB)
    vals = einsum('ban,bav->bnv', w, value)               (BS, NB, NV)
    out  = layernorm_over_NV(vals).transpose(0, 2, 1)     (BS, NV, NB)
    returns (out, w)

Sharding: data-parallel over BS across 8 cores (2 batches/core).

Numerics / speed choices (v2):
  - mm1 in bf16 with hi/lo split (error ~1e-6): s = khi@qhi + [khi;klo]@[qlo;qhi]
    streams 1 col/cycle on the PE (f32/f32r moving operands are 2-4x slower).
  - e = exp(s/8) stored fp16: mm2 runs at 1 col/cycle, and the softmax
    divide w = e * recip runs in DVE 2x_1P mode (all-16-bit operands).
  - w tiles are fp16; the gpsimd (SWDGE) DMA casts fp16->f32 on store.
  - softmax denominators: taken from the PE-transposed vals_aug tail
    (per-partition layout), reciprocal on 128 lanes, bounced through DRAM
    to partition-broadcast back as an fp16 row-replicated tile.
  - layernorm over NV via bn_stats/bn_aggr after PE transpose; unbiased
    std via exp(0.5*ln(var*64/63)) keeps ScalarE on one table set
    (natural_log_exp_and_others, pinned by filtering the table list).
  - the softmax denominator cancels in the layernorm (scale invariance).
"""

import sys

for _p in ("/opt/trn_rl_repo", "/opt/trn_rl_repo/concourse"):
    if _p not in sys.path:
        sys.path.insert(0, _p)

from contextlib import ExitStack

import ml_dtypes
import numpy as np

import concourse.bass as bass
import concourse.tile as tile
from concourse import bacc, mybir
from concourse.bass_utils import run_bass_kernel_spmd

BS, NA, NB, NK, NV = 16, 2048, 2048, 64, 64
NCORES = 8
BPC = BS // NCORES        # batches per core
NBH = NB // 2             # NB half processed at a time
NAC = NA // 128           # number of 128-row NA chunks
F32 = mybir.dt.float32
BF16 = mybir.dt.bfloat16
FP16 = mybir.dt.float16
AF = mybir.ActivationFunctionType

TRACE = False             # test.py flips this to profile
_cache = {}


class _PinnedBacc(bacc.Bacc):
    """Bacc that pins all ScalarE table loads to natural_log_exp_and_others.

    The default per-function choice alternates between exp_and_others (for
    Exp) and natural_log_exp_and_others (for Ln), reloading tables ~2.7us
    each time. Emptying the competing sets (list order/indices preserved)
    forces one resident set.
    """

    def insert_act_table_loads(self):
        from concourse.hw_specs import get_activation_tables
        from concourse.bass import _bass_rust

        has_activation = any(
            isinstance(i, mybir.InstActivation)
            for b in self.main_func.blocks
            for i in b.instructions
        )
        if not has_activation:
            return
        used = {AF.Exp, AF.Ln, AF.Copy, AF.Identity, AF.Square}
        tables = []
        for name, fns in get_activation_tables(self.m.arch).items():
            if name != "natural_log_exp_and_others":
                fns = fns - used
            tables.append((name, fns))
        _bass_rust.insert_act_table_loads(self, tables)


def _build():
    nc = _PinnedBacc("TRN2", target_bir_lowering=False, debug=False,
                     num_devices=NCORES)
    ks_d = nc.dram_tensor("ks", [BPC, 128, NA], BF16, kind="ExternalInput")
    qa_d = nc.dram_tensor("qa", [BPC, 128, NB], BF16, kind="ExternalInput")
    qb_d = nc.dram_tensor("qb", [BPC, 128, NB], BF16, kind="ExternalInput")
    va_d = nc.dram_tensor("va", [BPC, NAC, 128, 128], FP16,
                          kind="ExternalInput")
    id_d = nc.dram_tensor("ident", [128, 128], F32, kind="ExternalInput")
    w_d = nc.dram_tensor("w_out", [BPC, NA, NB], F32, kind="ExternalOutput")
    o_d = nc.dram_tensor("o_out", [BPC, NB, NV], F32, kind="ExternalOutput")

    with ExitStack() as ctx:
        tc = ctx.enter_context(tile.TileContext(nc))
        consts = ctx.enter_context(tc.tile_pool(name="consts", bufs=1))
        inp = ctx.enter_context(tc.tile_pool(name="inp", bufs=2))
        epool = ctx.enter_context(tc.tile_pool(name="e", bufs=32))
        wpool = ctx.enter_context(tc.tile_pool(name="w", bufs=8))
        tailp = ctx.enter_context(tc.tile_pool(name="tail", bufs=2))
        small = ctx.enter_context(tc.tile_pool(name="small", bufs=4))
        ps_s = ctx.enter_context(tc.tile_pool(name="ps_s", bufs=2, space="PSUM"))
        ps_acc = ctx.enter_context(tc.tile_pool(name="ps_acc", bufs=1, space="PSUM"))
        ps_tp = ctx.enter_context(tc.tile_pool(name="ps_tp", bufs=2, space="PSUM"))

        ident = consts.tile([128, 128], F32)
        nc.sync.dma_start(out=ident, in_=id_d[:, :])

        # prefetch all batches' inputs upfront so later-batch loads are
        # not queued behind tail DMAs on the sync queue
        loaded = []
        for b in range(BPC):
            ks = inp.tile([128, NA], BF16, tag="ks")
            nc.sync.dma_start(out=ks, in_=ks_d[b])
            qa = inp.tile([128, NB], BF16, tag="qa")
            nc.sync.dma_start(out=qa, in_=qa_d[b])
            qb = inp.tile([128, NB], BF16, tag="qb")
            nc.sync.dma_start(out=qb, in_=qb_d[b])
            va = inp.tile([128, NAC, 128], FP16, tag="va")
            nc.sync.dma_start(out=va, in_=va_d[b].rearrange("c p v -> p c v"))
            loaded.append((ks, qa, qb, va))

        for b in range(BPC):
            ks, qa, qb, va = loaded[b]

            for h in range(2):
                hs = h * NBH
                acc = ps_acc.tile([128, NBH], F32, tag="acc")
                es = []
                for a in range(NAC):
                    s = ps_s.tile([128, NBH], F32, tag="s")
                    for q in range(2):
                        sl = slice(hs + q * 512, hs + (q + 1) * 512)
                        nc.tensor.matmul(
                            s[:, q * 512:(q + 1) * 512],
                            lhsT=ks[:, a * 128:(a + 1) * 128],
                            rhs=qa[:, sl],
                            start=True, stop=False)
                        nc.tensor.matmul(
                            s[:, q * 512:(q + 1) * 512],
                            lhsT=ks[:, a * 128:(a + 1) * 128],
                            rhs=qb[:, sl],
                            start=False, stop=True)
                    e = epool.tile([128, NBH], FP16, tag="e")
                    nc.scalar.activation(e, s, AF.Exp, scale=float(1.0 / np.sqrt(NK)))
                    for q in range(2):
                        nc.tensor.matmul(
                            acc[:, q * 512:(q + 1) * 512],
                            lhsT=va[:, a, :],
                            rhs=e[:, q * 512:(q + 1) * 512],
                            start=(a == 0), stop=(a == NAC - 1))
                    es.append(e)

                # ---- softmax reciprocal: mm2 replicated the colsum on
                # partitions 64..127 (va cols 64..127 are all ones), so the
                # reciprocal runs on 64 lanes straight from PSUM and one
                # SBUF->SBUF DMA shifts it down to partitions 0..63.
                rb32 = small.tile([128, NBH], F32, tag="rb32")
                nc.vector.reciprocal(rb32[NV:128, :], acc[NV:128, :])
                rb = tailp.tile([128, NBH], FP16, tag="rb")
                nc.vector.tensor_copy(rb[NV:128, :], rb32[NV:128, :])
                nc.sync.dma_start(out=rb[0:NV, :], in_=rb[NV:2 * NV, :])

                # ---- w = e * recip (all fp16, DVE 2x), cast-store ----
                for g in range(4):
                    wt = wpool.tile([128, 4, NBH], FP16, tag="w")
                    for al in range(4):
                        nc.vector.tensor_mul(wt[:, al, :], es[g * 4 + al], rb)
                    nc.gpsimd.dma_start(
                        out=w_d[b, g * 512:(g + 1) * 512, hs:hs + NBH]
                            .rearrange("(a p) c -> p a c", p=128),
                        in_=wt)

                # ---- vals_aug tail: copy + transpose + stats ----
                va_s = tailp.tile([128, NBH], F32, tag="vas")
                nc.scalar.copy(va_s, acc)

                bigts = tailp.tile([128, 8, NV + 1], F32, tag="bigts")
                mvs = small.tile([128, 8, 2], F32, tag="mvs")
                for j in range(8):
                    tp = ps_tp.tile([128, 128], F32, tag="tp")
                    nc.tensor.transpose(tp, va_s[:, j * 128:(j + 1) * 128],
                                        ident)
                    nc.vector.tensor_copy(bigts[:, j, :], tp[:, 0:NV + 1])
                    st = small.tile([128, 6], F32, tag="st")
                    nc.vector.bn_stats(st, bigts[:, j, 0:NV])
                    nc.vector.bn_aggr(mvs[:, j, :], st)

                # ---- layernorm apply (d cancels by scale invariance) ----
                lt = small.tile([128, 8], F32, tag="lt")
                nc.scalar.activation(lt, mvs[:, :, 1], AF.Ln,
                                     scale=float(NV / (NV - 1)))
                sc = small.tile([128, 8], F32, tag="sc")
                nc.scalar.activation(sc, lt, AF.Exp, scale=-0.5)
                on = tailp.tile([128, 8, NV], F32, tag="on")
                for j in range(8):
                    nc.vector.tensor_scalar(
                        on[:, j, :], bigts[:, j, 0:NV],
                        scalar1=mvs[:, j, 0:1], scalar2=sc[:, j:j + 1],
                        op0=mybir.AluOpType.subtract,
                        op1=mybir.AluOpType.mult)
                nc.sync.dma_start(
                    out=o_d[b, hs:hs + NBH, :].rearrange("(j p) v -> p j v", p=128),
                    in_=on)

    nc.compile()
    return nc


def _prep_core_inputs(key, query, value, core):
    """Host-side shard + layout prep for one core (2 batches)."""
    b0 = core * BPC
    k = key[b0:b0 + BPC, :, 0, :]          # (BPC, NA, NK)
    q = query[b0:b0 + BPC, 0, :, :]        # (BPC, NB, NK)
    v = value[b0:b0 + BPC]                 # (BPC, NA, NV)
    kt = np.ascontiguousarray(k.transpose(0, 2, 1))   # (BPC, NK, NA) f32
    qt = np.ascontiguousarray(q.transpose(0, 2, 1))   # (BPC, NK, NB) f32

    khi = kt.astype(ml_dtypes.bfloat16)
    klo = (kt - khi.astype(np.float32)).astype(ml_dtypes.bfloat16)
    qhi = qt.astype(ml_dtypes.bfloat16)
    qlo = (qt - qhi.astype(np.float32)).astype(ml_dtypes.bfloat16)
    ks = np.concatenate([khi, klo], axis=1)           # (BPC, 128, NA)
    qa = np.concatenate([qhi, qhi], axis=1)           # (BPC, 128, NB)
    qb = np.concatenate(
        [qlo, np.zeros_like(qlo)], axis=1)            # (BPC, 128, NB)

    va = np.concatenate(
        [v, np.ones((BPC, NA, 128 - NV), np.float32)], axis=2)
    va = np.ascontiguousarray(
        va.reshape(BPC, NAC, 128, 128)).astype(np.float16)
    return {"ks": ks, "qa": qa, "qb": qb, "va": va,
            "ident": np.eye(128, dtype=np.float32)}


def kernel(key, query, value):
    key = np.asarray(key, dtype=np.float32)
    query = np.asarray(query, dtype=np.float32)
    value = np.asarray(value, dtype=np.float32)

    if "nc" not in _cache:
        _cache["nc"] = _build()
    nc = _cache["nc"]

    in_maps = [_prep_core_inputs(key, query, value, c) for c in range(NCORES)]
    res = run_bass_kernel_spmd(nc, in_maps, list(range(NCORES)), trace=TRACE)
    _cache["last_results"] = res

    w = np.concatenate([res.results[c]["w_out"] for c in range(NCORES)], axis=0)
    o_nbnv = np.concatenate([res.results[c]["o_out"] for c in range(NCORES)],
                            axis=0)                    # (BS, NB, NV)
    out = np.ascontiguousarray(o_nbnv.transpose(0, 2, 1))  # (BS, NV, NB)
    return out, w
